# revision 10
# baseline (speedup 1.0000x reference)
"""Trainium2 Bass kernel for the DiffRenderer problem.

Math (per grid cell): probs = softmax(grid_logits[r, c, :]); each cell's
28x14 tile = sum_n probs[n] * font[n]; tiles assembled into a (10752, 10752)
image.

Strategy (8 cores, data-parallel over grid rows — 48 rows per core):
  - Host shards grid_logits by row band, converts to fp16 (halves input
    HBM traffic; logits are N(0,1) so fp16 rounding is ~1e-3 relative on
    probs) and lays each band out as logitsT [69 chars, 36864 cells] so the
    char axis is on SBUF partitions.
  - The whole 5.1MB band preloads into SBUF via SWDGE triggers (gpsimd
    ring), throttled by pool WAR so head tiles' loads lead the ring.
  - Per load tile of 3072 cells: exps on ACT — tile 0 as four [69, 768]
    quarters (shortens the first-matmul ramp), later tiles as two
    [69, 1536] halves (fewer instructions, ~7us less ACT busy); then 12
    PSUM pairs of 2 matmuls (fp16, N=394 incl a ones column for the
    softmax denominator) into [128, 1024] (2 banks); one batched DVE
    reciprocal per pair; normalize+fp32->fp16 convert fused into the
    PSUM->SBUF move:
      * DVE pairs: ONE tensor_tensor per pair (reciprocal broadcast along
        the free axis via stride-0 AP) — amortizes the 120-cycle PSUM
        access latency over 1568 elements
      * ACT pairs: 2 copy-with-scale instructions
    ~53 ACT / ~91 DVE pairs balance the engines at ~100us busy each. The
    4-deep PSUM pipeline keeps the matmul->recip->evict->WAR chain off the
    critical path (a 2-deep variant with 4-bank groups serializes at
    ~2.2us/group and lands at 227us).
  - DMA out per half tile (1.2MB contiguous, SP ring); last tile splits the
    tail finer to shorten the drain.
  - Device output is the reference's soft_tiles data in a DMA-friendly
    permutation; the host performs the pure reindex to image form — the
    same transpose/reshape the reference itself performs after the math.
"""

import os
from contextlib import ExitStack

import numpy as np

os.environ.setdefault("MYCRO_LOCAL_CACHE", "1")

import concourse.bass as bass  # noqa: F401
import concourse.tile as tile
from concourse import bacc, mybir
from concourse.bass_utils import run_bass_kernel_spmd


def _install_ntff_hook_shim():
    """The image's antenv lacks axon_hooks, but run_bass_kernel_spmd imports
    it whenever BASS_TRACE is set. Provide the module and register the
    ctypes-based NTFF profile hook (degrades to no tracing if unavailable)."""
    import sys
    import types

    if "antenv.axon_hooks" in sys.modules:
        return
    try:
        import antenv
    except ImportError:
        return
    mod = types.ModuleType("antenv.axon_hooks")
    mod._hook = None
    mod.set_axon_ntff_profile_hook = lambda h: setattr(mod, "_hook", h)
    mod.get_axon_ntff_profile_hook = lambda: mod._hook
    sys.modules["antenv.axon_hooks"] = mod
    antenv.axon_hooks = mod
    try:
        from trn_agent_boot.trn_boot import _ntff_profile_via_ctypes

        hook = _ntff_profile_via_ctypes("/opt/axon/libaxon_pjrt.so")
        if hook is not None:
            mod.set_axon_ntff_profile_hook(hook)
    except Exception:
        pass


_install_ntff_hook_shim()

# Problem constants (hardcoded per harness contract)
ROWS, COLS, N_CHARS = 384, 768, 69
CH, CW = 28, 14
HW = CH * CW  # 392
NPAD = HW + 2  # col 392 = ones (softmax denom); 393 = pad (even free size)
N_CORES = 8
ROWS_PER_CORE = ROWS // N_CORES  # 48
CELLS = ROWS_PER_CORE * COLS  # 36864 cells per core
P = 128  # matmul output partitions (cells per chunk)
J = 24  # chunks per load tile
CT = P * J  # 3072 cells per load tile
T = CELLS // CT  # 12 load tiles per core
GW = 512  # psum cols per chunk slot (one 2KB bank)
HCT = CT // 2  # 1536 cells: one exp half
QCT = CT // 4  # 768 cells: one exp quarter (tile 0 ramp)
NPAIR = J // 2  # 12 chunk-pairs per load tile
F32 = mybir.dt.float32
F16 = mybir.dt.float16

# Stash of the last run's BassKernelResults (test.py reads exec_time_ns).
LAST_RESULTS = None
_CACHED_NC = None


def _act_pairs(t):
    """Chunk-pairs converted on ACT (scalar) — the rest go to DVE as one
    fused tensor_tensor each. 51 ACT / 93 DVE pairs balance the engines
    (ACT also runs the exps). Every tile ends on a DVE pair so each DMA
    segment's drain is one short instruction."""
    if t < 9:
        return (1, 4, 7, 10)
    return (1, 3, 5, 7, 9)


def _build_bass():
    nc = bacc.Bacc("TRN2", target_bir_lowering=False, debug=False,
                   num_devices=N_CORES)

    # [T, 69, CT]: the 6KB partition stride sprays SWDGE descriptors evenly
    # across all 16 DMA engines (a [69, CELLS] layout's 72KB stride aliases
    # the address-based queue spray onto a few queues)
    logits_h = nc.dram_tensor("logitsT", [T, N_CHARS, CT], F16,
                              kind="ExternalInput")
    fontb_h = nc.dram_tensor("fontb", [N_CHARS, NPAD], F16,
                             kind="ExternalInput")
    # out[t, p, j, :] holds soft_tiles for cell t*3072 + j*128 + p.
    out_h = nc.dram_tensor("out", [CELLS, HW], F16, kind="ExternalOutput")

    with tile.TileContext(nc) as tc, ExitStack() as ctx:
        # NOTE: tile dependencies are tracked per-TILE (readers wait on ALL
        # writers of a tile), so every independently produced/consumed piece
        # gets its own tile: quarter/half input and exp tiles, half-tile
        # output staging.
        singles = ctx.enter_context(tc.tile_pool(name="singles", bufs=1))
        lgq = ctx.enter_context(tc.tile_pool(name="lgq", bufs=4))
        # bufs=11: all 11 full tiles stay resident (loaded upfront over the
        # HWDGE sync ring, ~5MB lands by ~22us) so exps never wait on input
        lgp = ctx.enter_context(tc.tile_pool(name="lgp", bufs=11))
        exq = ctx.enter_context(tc.tile_pool(name="exq", bufs=4))
        expp = ctx.enter_context(tc.tile_pool(name="expp", bufs=4))
        outp = ctx.enter_context(tc.tile_pool(name="outp", bufs=6))
        rcpp = ctx.enter_context(tc.tile_pool(name="rcpp", bufs=8))
        # [128, 1024] = two PSUM banks per pair tile; 4 tiles = all 8 banks
        psp = ctx.enter_context(tc.tile_pool(name="psp", bufs=4,
                                             space="PSUM"))

        fontb_sb = singles.tile([N_CHARS, NPAD], F16)

        # Tile 0 arrives as four quarter tiles, emitted before everything
        # else — the first exp can start as soon as quarter 0 lands (~9us,
        # right after the preamble). All input goes over the HWDGE sync
        # ring: configs are ~0.6us each (no Q7 descriptor-gen serialization)
        # and the descriptors spread across all 16 DMA engines, so the whole
        # band is resident long before the back tiles need it.
        lg_v = logits_h[:]
        lg0q = [lgq.tile([N_CHARS, QCT], F16, name="lg0q")
                for _ in range(4)]
        for h in range(4):
            sl = slice(h * QCT, (h + 1) * QCT)
            nc.sync.dma_start(lg0q[h][:], lg_v[0][:, sl])
        nc.sync.dma_start(fontb_sb, fontb_h[:])
        lgs = [None] * T
        for t in range(1, T):
            lgs[t] = lgp.tile([N_CHARS, CT], F16, name="lg")
            nc.sync.dma_start(lgs[t][:], lg_v[t])

        out_v = out_h[:].rearrange("(t p j) f -> t p (j f)", p=P, j=J)

        # exp tiles: tile 0 = 4 quarters [69, 768]; tiles 1+ = 2 halves
        # [69, 1536]. eT(t, j) -> (tile, col0) for chunk j's 128 cells.
        eq0 = [None] * 4
        ehs = [[None, None] for _ in range(T)]
        for h in range(4):
            eq0[h] = exq.tile([N_CHARS, QCT], F16, name="eq0")
            nc.scalar.activation(eq0[h][:], lg0q[h][:],
                                 mybir.ActivationFunctionType.Exp)

        def eT(t, j):
            if t == 0:
                return eq0[j // 6], (j % 6) * P
            return ehs[t][j // 12], (j % 12) * P

        def issue_exp(t, h):
            ehs[t][h] = expp.tile([N_CHARS, HCT], F16, name="eh")
            nc.scalar.activation(ehs[t][h][:],
                                 lgs[t][:, h * HCT:(h + 1) * HCT],
                                 mybir.ActivationFunctionType.Exp)

        for t in range(T):
            acts = _act_pairs(t)
            # output staging: [chunks 0-12) and [12-24); the last tile
            # splits the tail as [12-20) + [20-24) to shorten the drain
            segs = [(0, 12), (12, 24)] if t < T - 1 else \
                [(0, 12), (12, 18), (18, 22), (22, 24)]
            seg_tiles = {}
            for (a, b) in segs:
                ot = outp.tile([P, 12 * HW], F16, name="ot")
                for j in range(a, b):
                    seg_tiles[j] = (ot, a, b)
            for q in range(NPAIR):
                psm = psp.tile([P, 2 * GW], F32)
                for k in range(2):
                    j = 2 * q + k
                    et, c0 = eT(t, j)
                    nc.tensor.matmul(psm[:, k * GW:k * GW + NPAD],
                                     et[:, c0:c0 + P],
                                     fontb_sb[:], start=True, stop=True)
                psm_v = psm[:].rearrange("p (c f) -> p c f", c=2)
                rc = rcpp.tile([P, 2], F32)
                rc_v = rc[:].rearrange("p (c f) -> p c f", c=2)
                nc.vector.reciprocal(rc_v, psm_v[:, :, HW:HW + 1])
                ot, a, _ = seg_tiles[2 * q]
                if q in acts:
                    for k in range(2):
                        j = 2 * q + k
                        c = j - a
                        nc.scalar.mul(ot[:, c * HW:(c + 1) * HW],
                                      psm[:, k * GW:k * GW + HW],
                                      rc[:, k:k + 1])
                else:
                    c = 2 * q - a
                    dst = ot[:, c * HW:(c + 2) * HW]
                    nc.vector.tensor_tensor(
                        dst.rearrange("p (c f) -> p c f", c=2),
                        psm_v[:, :, 0:HW],
                        rc_v.to_broadcast((P, 2, HW)),
                        mybir.AluOpType.mult)
                # exp halves for the next tile, interleaved between converts
                # so they never head-of-line block ACT's PSUM-freeing copies
                # for long
                if t + 1 < T and q in (2, 7):
                    issue_exp(t + 1, 0 if q == 2 else 1)
                # segment finished -> kick its DMA immediately
                for (a, b) in segs:
                    if 2 * q + 1 == b - 1:
                        ot = seg_tiles[a][0]
                        nc.sync.dma_start(
                            out_v[t][:, a * HW:b * HW],
                            ot[:, :(b - a) * HW])

    nc.compile()
    return nc


def kernel(grid_logits: np.ndarray, font: np.ndarray) -> np.ndarray:
    global LAST_RESULTS, _CACHED_NC
    grid_logits = np.asarray(grid_logits, dtype=np.float32)
    font = np.asarray(font, dtype=np.float32)
    assert grid_logits.shape == (ROWS, COLS, N_CHARS)
    assert font.shape == (N_CHARS, CH, CW)

    fontb = np.zeros((N_CHARS, NPAD), dtype=np.float32)
    fontb[:, :HW] = font.reshape(N_CHARS, HW)
    fontb[:, HW] = 1.0
    fontb = fontb.astype(np.float16)

    # (69, 384, 768) with chars leading: one big transpose, then per-core
    # contiguous band slices, downcast to fp16 for half the load traffic
    glT = grid_logits.transpose(2, 0, 1)

    in_maps = []
    for k in range(N_CORES):
        band = glT[:, k * ROWS_PER_CORE:(k + 1) * ROWS_PER_CORE, :]
        bandc = np.ascontiguousarray(band, dtype=np.float16)
        bandc = bandc.reshape(N_CHARS, T, CT)
        in_maps.append({
            "logitsT": np.ascontiguousarray(bandc.transpose(1, 0, 2)),
            "fontb": fontb,
        })

    if _CACHED_NC is None:
        _CACHED_NC = _build_bass()

    res = run_bass_kernel_spmd(_CACHED_NC, in_maps,
                               core_ids=list(range(N_CORES)))
    LAST_RESULTS = res

    img = np.empty((ROWS * CH, COLS * CW), dtype=np.float32)
    band_h = ROWS_PER_CORE * CH  # 1344
    for k in range(N_CORES):
        arr = res.results[k]["out"].reshape(T, P, J, CH, CW)
        # [t, p, j] holds cell t*3072 + j*128 + p -> reorder to cell-major
        cells = arr.transpose(0, 2, 1, 3, 4).reshape(
            ROWS_PER_CORE, COLS, CH, CW)
        img[k * band_h:(k + 1) * band_h] = (
            cells.transpose(0, 2, 1, 3).reshape(band_h, COLS * CW))
    return img[None, None]


# revision 14
# speedup vs baseline: 1.1676x; 1.1676x over previous
"""Trainium2 Bass kernel for the DiffRenderer problem.

Math (per grid cell): probs = softmax(grid_logits[r, c, :]); each cell's
28x14 tile = sum_n probs[n] * font[n]; tiles assembled into a (10752, 10752)
image.

Strategy (8 cores, data-parallel over grid rows — 48 rows per core):
  - Host shards grid_logits by row band, converts to fp16 (halves input
    HBM traffic; logits are N(0,1) so fp16 rounding is ~1e-3 relative on
    probs) and lays each band out as logitsT [69 chars, 36864 cells] so the
    char axis is on SBUF partitions.
  - The whole 5.1MB band preloads into SBUF via SWDGE triggers (gpsimd
    ring), throttled by pool WAR so head tiles' loads lead the ring.
  - Per load tile of 3072 cells: exps on ACT — tile 0 as four [69, 768]
    quarters (shortens the first-matmul ramp), later tiles as two
    [69, 1536] halves (fewer instructions, ~7us less ACT busy); then 12
    PSUM pairs of 2 matmuls (fp16, N=394 incl a ones column for the
    softmax denominator) into [128, 1024] (2 banks); one batched DVE
    reciprocal per pair; normalize+fp32->fp16 convert fused into the
    PSUM->SBUF move:
      * DVE pairs: ONE tensor_tensor per pair (reciprocal broadcast along
        the free axis via stride-0 AP) — amortizes the 120-cycle PSUM
        access latency over 1568 elements
      * ACT pairs: 2 copy-with-scale instructions
    ~53 ACT / ~91 DVE pairs balance the engines at ~100us busy each. The
    4-deep PSUM pipeline keeps the matmul->recip->evict->WAR chain off the
    critical path (a 2-deep variant with 4-bank groups serializes at
    ~2.2us/group and lands at 227us).
  - DMA out per half tile (1.2MB contiguous, SP ring); last tile splits the
    tail finer to shorten the drain.
  - Device output is the reference's soft_tiles data in a DMA-friendly
    permutation; the host performs the pure reindex to image form — the
    same transpose/reshape the reference itself performs after the math.
"""

import os
from contextlib import ExitStack

import numpy as np

os.environ.setdefault("MYCRO_LOCAL_CACHE", "1")

import concourse.bass as bass  # noqa: F401
import concourse.tile as tile
from concourse import bacc, mybir
from concourse.bass_utils import run_bass_kernel_spmd


def _install_ntff_hook_shim():
    """The image's antenv lacks axon_hooks, but run_bass_kernel_spmd imports
    it whenever BASS_TRACE is set. Provide the module and register the
    ctypes-based NTFF profile hook (degrades to no tracing if unavailable)."""
    import sys
    import types

    if "antenv.axon_hooks" in sys.modules:
        return
    try:
        import antenv
    except ImportError:
        return
    mod = types.ModuleType("antenv.axon_hooks")
    mod._hook = None
    mod.set_axon_ntff_profile_hook = lambda h: setattr(mod, "_hook", h)
    mod.get_axon_ntff_profile_hook = lambda: mod._hook
    sys.modules["antenv.axon_hooks"] = mod
    antenv.axon_hooks = mod
    try:
        from trn_agent_boot.trn_boot import _ntff_profile_via_ctypes

        hook = _ntff_profile_via_ctypes("/opt/axon/libaxon_pjrt.so")
        if hook is not None:
            mod.set_axon_ntff_profile_hook(hook)
    except Exception:
        pass


_install_ntff_hook_shim()

# Problem constants (hardcoded per harness contract)
ROWS, COLS, N_CHARS = 384, 768, 69
CH, CW = 28, 14
HW = CH * CW  # 392
NPAD = HW + 2  # col 392 = ones (softmax denom); 393 = pad (even free size)
N_CORES = 8
ROWS_PER_CORE = ROWS // N_CORES  # 48
CELLS = ROWS_PER_CORE * COLS  # 36864 cells per core
P = 128  # matmul output partitions (cells per chunk)
J = 24  # chunks per load tile
CT = P * J  # 3072 cells per load tile
T = CELLS // CT  # 12 load tiles per core
GW = 512  # psum cols per chunk slot (one 2KB bank)
HCT = CT // 2  # 1536 cells: one exp half
QCT = CT // 4  # 768 cells: one exp quarter (tile 0 ramp)
NPAIR = J // 2  # 12 chunk-pairs per load tile
F32 = mybir.dt.float32
F16 = mybir.dt.float16

# Stash of the last run's BassKernelResults (test.py reads exec_time_ns).
LAST_RESULTS = None
_CACHED_NC = None


def _act_pairs(t):
    """Chunk-pairs converted on ACT (scalar) — the rest go to DVE as one
    fused tensor_tensor each. 51 ACT / 93 DVE pairs balance the engines
    (ACT also runs the exps). ACT pairs sit in the BACK half of each tile:
    both next-tile exps issue at pair 0, so ACT exps while DVE alone drains
    pairs 0-3 — the exp never delays an ACT copy that PE's psum WAR is
    waiting on (interleaved exps cost ~3 convoy stalls/tile, ~17us/core).
    Every tile ends on a DVE pair so the final DMA segment's drain is one
    short instruction."""
    if t < 9:
        return (4, 6, 8, 10)
    return (2, 4, 6, 8, 10)


def _build_bass():
    nc = bacc.Bacc("TRN2", target_bir_lowering=False, debug=False,
                   num_devices=N_CORES)

    # [T, 69, CT]: the 6KB partition stride sprays SWDGE descriptors evenly
    # across all 16 DMA engines (a [69, CELLS] layout's 72KB stride aliases
    # the address-based queue spray onto a few queues)
    logits_h = nc.dram_tensor("logitsT", [T, N_CHARS, CT], F16,
                              kind="ExternalInput")
    fontb_h = nc.dram_tensor("fontb", [N_CHARS, NPAD], F16,
                             kind="ExternalInput")
    # out[t, p, j, :] holds soft_tiles for cell t*3072 + j*128 + p.
    out_h = nc.dram_tensor("out", [CELLS, HW], F16, kind="ExternalOutput")

    with tile.TileContext(nc) as tc, ExitStack() as ctx:
        # NOTE: tile dependencies are tracked per-TILE (readers wait on ALL
        # writers of a tile), so every independently produced/consumed piece
        # gets its own tile: quarter/half input and exp tiles, half-tile
        # output staging.
        singles = ctx.enter_context(tc.tile_pool(name="singles", bufs=1))
        lgq = ctx.enter_context(tc.tile_pool(name="lgq", bufs=4))
        # bufs=11: all 11 full tiles stay resident (loaded upfront over the
        # HWDGE sync ring, ~5MB lands by ~22us) so exps never wait on input
        lgp = ctx.enter_context(tc.tile_pool(name="lgp", bufs=11))
        exq = ctx.enter_context(tc.tile_pool(name="exq", bufs=4))
        expp = ctx.enter_context(tc.tile_pool(name="expp", bufs=4))
        outp = ctx.enter_context(tc.tile_pool(name="outp", bufs=6))
        rcpp = ctx.enter_context(tc.tile_pool(name="rcpp", bufs=8))
        # [128, 1024] = two PSUM banks per pair tile; 4 tiles = all 8 banks
        psp = ctx.enter_context(tc.tile_pool(name="psp", bufs=4,
                                             space="PSUM"))

        fontb_sb = singles.tile([N_CHARS, NPAD], F16)

        # Tile 0 arrives as four quarter tiles on the SWDGE (gpsimd) ring,
        # emitted before everything else so their descriptors lead the ring
        # and spread across all 16 DMA engines — the first exp can start as
        # soon as quarter 0 lands (~10us). Input must stay on the SWDGE
        # ring: the HWDGE address-based queue spray aliases these 6KB-stride
        # rows onto DMA queues 0-2, starving the output path (a tried HWDGE
        # variant lost 27us to one 26us whole-convoy stall).
        lg_v = logits_h[:]
        lg0q = [lgq.tile([N_CHARS, QCT], F16, name="lg0q")
                for _ in range(4)]
        for h in range(4):
            sl = slice(h * QCT, (h + 1) * QCT)
            nc.gpsimd.dma_start(lg0q[h][:], lg_v[0][:, sl])
        nc.sync.dma_start(fontb_sb, fontb_h[:])
        lgs = [None] * T
        for t in range(1, T):
            lgs[t] = lgp.tile([N_CHARS, CT], F16, name="lg")
            nc.gpsimd.dma_start(lgs[t][:], lg_v[t])

        out_v = out_h[:].rearrange("(t p j) f -> t p (j f)", p=P, j=J)

        # exp tiles: tile 0 = 4 quarters [69, 768]; tiles 1+ = 2 halves
        # [69, 1536]. eT(t, j) -> (tile, col0) for chunk j's 128 cells.
        eq0 = [None] * 4
        ehs = [[None, None] for _ in range(T)]
        for h in range(4):
            eq0[h] = exq.tile([N_CHARS, QCT], F16, name="eq0")
            nc.scalar.activation(eq0[h][:], lg0q[h][:],
                                 mybir.ActivationFunctionType.Exp)

        def eT(t, j):
            if t == 0:
                return eq0[j // 6], (j % 6) * P
            return ehs[t][j // 12], (j % 12) * P

        def issue_exp(t, h):
            ehs[t][h] = expp.tile([N_CHARS, HCT], F16, name="eh")
            nc.scalar.activation(ehs[t][h][:],
                                 lgs[t][:, h * HCT:(h + 1) * HCT],
                                 mybir.ActivationFunctionType.Exp)

        for t in range(T):
            acts = _act_pairs(t)
            # output staging: [chunks 0-12) and [12-24); the last tile
            # splits the tail as [12-20) + [20-24) to shorten the drain
            segs = [(0, 12), (12, 24)] if t < T - 1 else \
                [(0, 12), (12, 18), (18, 22), (22, 24)]
            seg_tiles = {}
            for (a, b) in segs:
                ot = outp.tile([P, 12 * HW], F16, name="ot")
                for j in range(a, b):
                    seg_tiles[j] = (ot, a, b)
            for q in range(NPAIR):
                psm = psp.tile([P, 2 * GW], F32)
                for k in range(2):
                    j = 2 * q + k
                    et, c0 = eT(t, j)
                    nc.tensor.matmul(psm[:, k * GW:k * GW + NPAD],
                                     et[:, c0:c0 + P],
                                     fontb_sb[:], start=True, stop=True)
                psm_v = psm[:].rearrange("p (c f) -> p c f", c=2)
                rc = rcpp.tile([P, 2], F32)
                rc_v = rc[:].rearrange("p (c f) -> p c f", c=2)
                nc.vector.reciprocal(rc_v, psm_v[:, :, HW:HW + 1])
                ot, a, _ = seg_tiles[2 * q]
                if q in acts:
                    for k in range(2):
                        j = 2 * q + k
                        c = j - a
                        nc.scalar.mul(ot[:, c * HW:(c + 1) * HW],
                                      psm[:, k * GW:k * GW + HW],
                                      rc[:, k:k + 1])
                else:
                    c = 2 * q - a
                    dst = ot[:, c * HW:(c + 2) * HW]
                    nc.vector.tensor_tensor(
                        dst.rearrange("p (c f) -> p c f", c=2),
                        psm_v[:, :, 0:HW],
                        rc_v.to_broadcast((P, 2, HW)),
                        mybir.AluOpType.mult)
                # both exp halves for the next tile go at pair 0, while all
                # of this tile's early pairs drain on DVE — ACT has no psum
                # work the convoy could block on until pair 4
                if t + 1 < T and q == 0:
                    issue_exp(t + 1, 0)
                    issue_exp(t + 1, 1)
                # segment finished -> kick its DMA immediately
                for (a, b) in segs:
                    if 2 * q + 1 == b - 1:
                        ot = seg_tiles[a][0]
                        nc.sync.dma_start(
                            out_v[t][:, a * HW:b * HW],
                            ot[:, :(b - a) * HW])

    nc.compile()
    return nc


def kernel(grid_logits: np.ndarray, font: np.ndarray) -> np.ndarray:
    global LAST_RESULTS, _CACHED_NC
    grid_logits = np.asarray(grid_logits, dtype=np.float32)
    font = np.asarray(font, dtype=np.float32)
    assert grid_logits.shape == (ROWS, COLS, N_CHARS)
    assert font.shape == (N_CHARS, CH, CW)

    fontb = np.zeros((N_CHARS, NPAD), dtype=np.float32)
    fontb[:, :HW] = font.reshape(N_CHARS, HW)
    fontb[:, HW] = 1.0
    fontb = fontb.astype(np.float16)

    # (69, 384, 768) with chars leading: one big transpose, then per-core
    # contiguous band slices, downcast to fp16 for half the load traffic
    glT = grid_logits.transpose(2, 0, 1)

    in_maps = []
    for k in range(N_CORES):
        band = glT[:, k * ROWS_PER_CORE:(k + 1) * ROWS_PER_CORE, :]
        bandc = np.ascontiguousarray(band, dtype=np.float16)
        bandc = bandc.reshape(N_CHARS, T, CT)
        in_maps.append({
            "logitsT": np.ascontiguousarray(bandc.transpose(1, 0, 2)),
            "fontb": fontb,
        })

    if _CACHED_NC is None:
        _CACHED_NC = _build_bass()

    res = run_bass_kernel_spmd(_CACHED_NC, in_maps,
                               core_ids=list(range(N_CORES)))
    LAST_RESULTS = res

    img = np.empty((ROWS * CH, COLS * CW), dtype=np.float32)
    band_h = ROWS_PER_CORE * CH  # 1344
    for k in range(N_CORES):
        arr = res.results[k]["out"].reshape(T, P, J, CH, CW)
        # [t, p, j] holds cell t*3072 + j*128 + p -> reorder to cell-major
        cells = arr.transpose(0, 2, 1, 3, 4).reshape(
            ROWS_PER_CORE, COLS, CH, CW)
        img[k * band_h:(k + 1) * band_h] = (
            cells.transpose(0, 2, 1, 3).reshape(band_h, COLS * CW))
    return img[None, None]


# revision 18
# speedup vs baseline: 1.1706x; 1.0026x over previous
"""Trainium2 Bass kernel for the DiffRenderer problem.

Math (per grid cell): probs = softmax(grid_logits[r, c, :]); each cell's
28x14 tile = sum_n probs[n] * font[n]; tiles assembled into a (10752, 10752)
image.

Strategy (8 cores, data-parallel over grid rows — 48 rows per core):
  - Host shards grid_logits by row band, converts to fp16 (halves input
    HBM traffic; logits are N(0,1) so fp16 rounding is ~1e-3 relative on
    probs) and lays each band out as logitsT [69 chars, 36864 cells] so the
    char axis is on SBUF partitions.
  - The whole 5.1MB band preloads into SBUF via SWDGE triggers (gpsimd
    ring), throttled by pool WAR so head tiles' loads lead the ring.
  - Per load tile of 3072 cells: exps on ACT — tile 0 as four [69, 768]
    quarters (shortens the first-matmul ramp), later tiles as two
    [69, 1536] halves (fewer instructions, ~7us less ACT busy); then 12
    PSUM pairs of 2 matmuls (fp16, N=394 incl a ones column for the
    softmax denominator) into [128, 1024] (2 banks); one batched DVE
    reciprocal per pair; normalize+fp32->fp16 convert fused into the
    PSUM->SBUF move:
      * DVE pairs: ONE tensor_tensor per pair (reciprocal broadcast along
        the free axis via stride-0 AP) — amortizes the 120-cycle PSUM
        access latency over 1568 elements
      * ACT pairs: 2 copy-with-scale instructions
    ~53 ACT / ~91 DVE pairs balance the engines at ~100us busy each. The
    4-deep PSUM pipeline keeps the matmul->recip->evict->WAR chain off the
    critical path (a 2-deep variant with 4-bank groups serializes at
    ~2.2us/group and lands at 227us).
  - DMA out per half tile (1.2MB contiguous, SP ring); last tile splits the
    tail finer to shorten the drain.
  - Device output is the reference's soft_tiles data in a DMA-friendly
    permutation; the host performs the pure reindex to image form — the
    same transpose/reshape the reference itself performs after the math.
"""

import os
from contextlib import ExitStack

import numpy as np

os.environ.setdefault("MYCRO_LOCAL_CACHE", "1")

import concourse.bass as bass  # noqa: F401
import concourse.tile as tile
from concourse import bacc, mybir
from concourse.bass_utils import run_bass_kernel_spmd


def _install_ntff_hook_shim():
    """The image's antenv lacks axon_hooks, but run_bass_kernel_spmd imports
    it whenever BASS_TRACE is set. Provide the module and register the
    ctypes-based NTFF profile hook (degrades to no tracing if unavailable)."""
    import sys
    import types

    if "antenv.axon_hooks" in sys.modules:
        return
    try:
        import antenv
    except ImportError:
        return
    mod = types.ModuleType("antenv.axon_hooks")
    mod._hook = None
    mod.set_axon_ntff_profile_hook = lambda h: setattr(mod, "_hook", h)
    mod.get_axon_ntff_profile_hook = lambda: mod._hook
    sys.modules["antenv.axon_hooks"] = mod
    antenv.axon_hooks = mod
    try:
        from trn_agent_boot.trn_boot import _ntff_profile_via_ctypes

        hook = _ntff_profile_via_ctypes("/opt/axon/libaxon_pjrt.so")
        if hook is not None:
            mod.set_axon_ntff_profile_hook(hook)
    except Exception:
        pass


_install_ntff_hook_shim()

# Problem constants (hardcoded per harness contract)
ROWS, COLS, N_CHARS = 384, 768, 69
CH, CW = 28, 14
HW = CH * CW  # 392
NPAD = HW + 2  # col 392 = ones (softmax denom); 393 = pad (even free size)
N_CORES = 8
ROWS_PER_CORE = ROWS // N_CORES  # 48
CELLS = ROWS_PER_CORE * COLS  # 36864 cells per core
P = 128  # matmul output partitions (cells per chunk)
J = 24  # chunks per load tile
CT = P * J  # 3072 cells per load tile
T = CELLS // CT  # 12 load tiles per core
GW = 512  # psum cols per chunk slot (one 2KB bank)
HCT = CT // 2  # 1536 cells: one exp half
QCT = CT // 4  # 768 cells: one exp quarter (tile 0 ramp)
NPAIR = J // 2  # 12 chunk-pairs per load tile
F32 = mybir.dt.float32
F16 = mybir.dt.float16

# Stash of the last run's BassKernelResults (test.py reads exec_time_ns).
LAST_RESULTS = None
_CACHED_NC = None


def _act_pairs(t):
    """Chunk-pairs converted on ACT (scalar) — the rest go to DVE as one
    fused tensor_tensor each. 51 ACT / 93 DVE pairs balance the engines
    (ACT also runs the exps). ACT pairs sit in the BACK half of each tile:
    both next-tile exps issue at pair 0, so ACT exps while DVE alone drains
    pairs 0-3 — the exp never delays an ACT copy that PE's psum WAR is
    waiting on (interleaved exps cost ~3 convoy stalls/tile, ~17us/core).
    Every tile ends on a DVE pair so the final DMA segment's drain is one
    short instruction."""
    if t < 9:
        return (4, 6, 8, 10)
    return (2, 4, 6, 8, 10)


def _build_bass():
    nc = bacc.Bacc("TRN2", target_bir_lowering=False, debug=False,
                   num_devices=N_CORES)

    # [T, 69, CT]: the 6KB partition stride sprays SWDGE descriptors evenly
    # across all 16 DMA engines (a [69, CELLS] layout's 72KB stride aliases
    # the address-based queue spray onto a few queues)
    logits_h = nc.dram_tensor("logitsT", [T, N_CHARS, CT], F16,
                              kind="ExternalInput")
    fontb_h = nc.dram_tensor("fontb", [N_CHARS, NPAD], F16,
                             kind="ExternalInput")
    # out[t, p, j, :] holds soft_tiles for cell t*3072 + j*128 + p.
    out_h = nc.dram_tensor("out", [CELLS, HW], F16, kind="ExternalOutput")

    with tile.TileContext(nc) as tc, ExitStack() as ctx:
        # NOTE: tile dependencies are tracked per-TILE (readers wait on ALL
        # writers of a tile), so every independently produced/consumed piece
        # gets its own tile: quarter/half input and exp tiles, half-tile
        # output staging.
        singles = ctx.enter_context(tc.tile_pool(name="singles", bufs=1))
        lgq = ctx.enter_context(tc.tile_pool(name="lgq", bufs=4))
        # bufs=22: all 22 half-tiles stay resident (SWDGE-loaded upfront,
        # whole band lands by ~35us) so exps never wait on input; halves
        # rather than fulls so tile 1's first half lands ~3.5us sooner
        lgp = ctx.enter_context(tc.tile_pool(name="lgp", bufs=22))
        exq = ctx.enter_context(tc.tile_pool(name="exq", bufs=4))
        expp = ctx.enter_context(tc.tile_pool(name="expp", bufs=6))
        outp = ctx.enter_context(tc.tile_pool(name="outp", bufs=6))
        rcpp = ctx.enter_context(tc.tile_pool(name="rcpp", bufs=12))
        # [128, 1024] = two PSUM banks per pair tile; 4 tiles = all 8 banks
        psp = ctx.enter_context(tc.tile_pool(name="psp", bufs=4,
                                             space="PSUM"))

        fontb_sb = singles.tile([N_CHARS, NPAD], F16)

        # Tile 0 arrives as four quarter tiles on the SWDGE (gpsimd) ring,
        # emitted before everything else so their descriptors lead the ring
        # and spread across all 16 DMA engines — the first exp can start as
        # soon as quarter 0 lands (~10us). Input must stay on the SWDGE
        # ring: the HWDGE address-based queue spray aliases these 6KB-stride
        # rows onto DMA queues 0-2, starving the output path (a tried HWDGE
        # variant lost 27us to one 26us whole-convoy stall).
        lg_v = logits_h[:]
        lg0q = [lgq.tile([N_CHARS, QCT], F16, name="lg0q")
                for _ in range(4)]
        for h in range(4):
            sl = slice(h * QCT, (h + 1) * QCT)
            nc.gpsimd.dma_start(lg0q[h][:], lg_v[0][:, sl])
        nc.sync.dma_start(fontb_sb, fontb_h[:])
        lgh = [[None, None] for _ in range(T)]
        for t in range(1, T):
            for h in range(2):
                lgh[t][h] = lgp.tile([N_CHARS, HCT], F16, name="lg")
                nc.gpsimd.dma_start(lgh[t][h][:],
                                    lg_v[t][:, h * HCT:(h + 1) * HCT])

        out_v = out_h[:].rearrange("(t p j) f -> t p (j f)", p=P, j=J)

        # exp tiles: tile 0 = 4 quarters [69, 768]; tiles 1+ = 2 halves
        # [69, 1536]. eT(t, j) -> (tile, col0) for chunk j's 128 cells.
        eq0 = [None] * 4
        ehs = [[None, None] for _ in range(T)]
        for h in range(4):
            eq0[h] = exq.tile([N_CHARS, QCT], F16, name="eq0")
            nc.scalar.activation(eq0[h][:], lg0q[h][:],
                                 mybir.ActivationFunctionType.Exp)

        def eT(t, j):
            if t == 0:
                return eq0[j // 6], (j % 6) * P
            return ehs[t][j // 12], (j % 12) * P

        def issue_exp(t, h):
            ehs[t][h] = expp.tile([N_CHARS, HCT], F16, name="eh")
            nc.scalar.activation(ehs[t][h][:], lgh[t][h][:],
                                 mybir.ActivationFunctionType.Exp)

        for t in range(T):
            acts = _act_pairs(t)
            # output staging: [chunks 0-12) and [12-24); the last tile
            # splits the tail as [12-20) + [20-24) to shorten the drain
            segs = [(0, 12), (12, 24)] if t < T - 1 else \
                [(0, 12), (12, 18), (18, 22), (22, 24)]
            seg_tiles = {}
            for (a, b) in segs:
                ot = outp.tile([P, 12 * HW], F16, name="ot")
                for j in range(a, b):
                    seg_tiles[j] = (ot, a, b)
            for q in range(NPAIR):
                psm = psp.tile([P, 2 * GW], F32)
                for k in range(2):
                    j = 2 * q + k
                    et, c0 = eT(t, j)
                    nc.tensor.matmul(psm[:, k * GW:k * GW + NPAD],
                                     et[:, c0:c0 + P],
                                     fontb_sb[:], start=True, stop=True)
                psm_v = psm[:].rearrange("p (c f) -> p c f", c=2)
                rc = rcpp.tile([P, 2], F32)
                rc_v = rc[:].rearrange("p (c f) -> p c f", c=2)
                nc.vector.reciprocal(rc_v, psm_v[:, :, HW:HW + 1])
                ot, a, _ = seg_tiles[2 * q]
                if q in acts:
                    for k in range(2):
                        j = 2 * q + k
                        c = j - a
                        nc.scalar.mul(ot[:, c * HW:(c + 1) * HW],
                                      psm[:, k * GW:k * GW + HW],
                                      rc[:, k:k + 1])
                else:
                    c = 2 * q - a
                    dst = ot[:, c * HW:(c + 2) * HW]
                    nc.vector.tensor_tensor(
                        dst.rearrange("p (c f) -> p c f", c=2),
                        psm_v[:, :, 0:HW],
                        rc_v.to_broadcast((P, 2, HW)),
                        mybir.AluOpType.mult)
                # both exp halves for the next tile go at pair 0, while all
                # of this tile's early pairs drain on DVE — ACT has no psum
                # work the convoy could block on until pair 4
                if t + 1 < T and q == 0:
                    issue_exp(t + 1, 0)
                    issue_exp(t + 1, 1)
                # segment finished -> kick its DMA immediately
                for (a, b) in segs:
                    if 2 * q + 1 == b - 1:
                        ot = seg_tiles[a][0]
                        nc.sync.dma_start(
                            out_v[t][:, a * HW:b * HW],
                            ot[:, :(b - a) * HW])

    nc.compile()
    return nc


def kernel(grid_logits: np.ndarray, font: np.ndarray) -> np.ndarray:
    global LAST_RESULTS, _CACHED_NC
    grid_logits = np.asarray(grid_logits, dtype=np.float32)
    font = np.asarray(font, dtype=np.float32)
    assert grid_logits.shape == (ROWS, COLS, N_CHARS)
    assert font.shape == (N_CHARS, CH, CW)

    fontb = np.zeros((N_CHARS, NPAD), dtype=np.float32)
    fontb[:, :HW] = font.reshape(N_CHARS, HW)
    fontb[:, HW] = 1.0
    fontb = fontb.astype(np.float16)

    # (69, 384, 768) with chars leading: one big transpose, then per-core
    # contiguous band slices, downcast to fp16 for half the load traffic
    glT = grid_logits.transpose(2, 0, 1)

    in_maps = []
    for k in range(N_CORES):
        band = glT[:, k * ROWS_PER_CORE:(k + 1) * ROWS_PER_CORE, :]
        bandc = np.ascontiguousarray(band, dtype=np.float16)
        bandc = bandc.reshape(N_CHARS, T, CT)
        in_maps.append({
            "logitsT": np.ascontiguousarray(bandc.transpose(1, 0, 2)),
            "fontb": fontb,
        })

    if _CACHED_NC is None:
        _CACHED_NC = _build_bass()

    res = run_bass_kernel_spmd(_CACHED_NC, in_maps,
                               core_ids=list(range(N_CORES)))
    LAST_RESULTS = res

    img = np.empty((ROWS * CH, COLS * CW), dtype=np.float32)
    band_h = ROWS_PER_CORE * CH  # 1344
    for k in range(N_CORES):
        arr = res.results[k]["out"].reshape(T, P, J, CH, CW)
        # [t, p, j] holds cell t*3072 + j*128 + p -> reorder to cell-major
        cells = arr.transpose(0, 2, 1, 3, 4).reshape(
            ROWS_PER_CORE, COLS, CH, CW)
        img[k * band_h:(k + 1) * band_h] = (
            cells.transpose(0, 2, 1, 3).reshape(band_h, COLS * CW))
    return img[None, None]
